# revision 8
# baseline (speedup 1.0000x reference)
"""ComplEx scoring kernel for 8 Trainium2 NeuronCores.

Math: score[b, e] = Re(<h_b * r_b, conj(ent_e)>) with h = ent_emb[triples[:,0]],
r = rel_emb[triples[:,1]].  Writing ans_b = concat(re_h*re_r - im_h*im_r,
re_h*im_r + im_h*re_r) (shape [B, 512]), the score is exactly
score = ans @ ent_emb.T  — one [1024, 512] x [512, 200000] GEMM.

Strategy (vocab/tensor parallel along the entity axis):
  - host: tiny gather + complex multiply -> ans  (microseconds)
  - shard ent_emb rows 8 ways (25000/core, zero-padded to 25088 = 49*512),
    pre-transposed + cast on host so the device streams contiguous
    [K=512, E] tiles
  - each core: score_shard[1024, 25088] = ansT.T @ entT on the PE array.
    The kernel is PE-bound, so 10 of the 49 column tiles run in fp8e4
    DoubleRow perf mode (2 fp8 K-rows per cycle — halves those columns'
    matmul time; ~3.8% quantization noise on 20% of columns puts the
    global rel err at ~1.7e-2, inside the 2e-2 budget).  The fp8 tiles
    sit at the two ends: 3 tiles open group 0 (lighter warmup stream)
    and group 6 is all fp8 (its blocks run after the input queues have
    drained, where the copyback engines have slack).  bf16 elsewhere,
    fp32 PSUM accumulate everywhere.
  - DMA plumbing: inputs ride the SP hardware queue (warmup alternates
    SP/Act), outputs mostly the Act queue, so prefetch and drain never
    serialize; the first and last blocks stream per-512-column outputs
    so warmup overlaps and the post-matmul tail is ~1 us
  - host: concatenate the 8 column slabs, unscale, drop padding
"""

import numpy as np
import ml_dtypes

NCORES = 8
NUM_ENT = 200000
EMB = 512
B = 1024
SHARD = NUM_ENT // NCORES      # 25000 entities per core
NTILE = 512                    # matmul moving free dim == one PSUM bank
TPG = 7                        # 512-tiles per DMA group
GN = NTILE * TPG               # 3584 entities per group
NGROUPS = 7
SHARD_PAD = GN * NGROUPS       # 25088
KCH = EMB // 128               # 4 contraction chunks
KPAIRS = 2                     # 2 x (K=256) DoubleRow steps cover K=512
MCH = B // 128                 # 8 batch chunks
T8A = 3                        # leading tiles of group 0 in fp8
NBF0 = TPG - T8A               # group 0's bf16 tiles
A8_COLS = T8A * NTILE          # 1536 fp8 columns at the front
BF_COLS = NBF0 * NTILE + 5 * GN   # bf16 columns: g0 tail + groups 1-5

_NC = None

# score values are ~1e-5 — subnormal in fp16.  Pre-scaling ans by 2**16 on
# the host puts the device-side scores in fp16's normal range, so the output
# can be stored/DMA'd as fp16 (half the write traffic); the host unscales.
OUT_SCALE = 2.0 ** 16
# fp8 operands get extra power-of-2 gain to sit comfortably inside e4m3's
# +-240 range: ans * 2**17 (abs max ~190), ent * 2**11 (abs max ~36).  The
# fp8 columns' scores come out 2**12 hotter than the bf16 ones (max ~2e4,
# still inside f16); assemble() divides that back out.
ANS8_SCALE = 2.0 ** 17
ENT8_SCALE = 2.0 ** 11
FP8_EXTRA = ANS8_SCALE * ENT8_SCALE / OUT_SCALE


def _build_nc():
    import concourse.bacc as bacc
    import concourse.bass as bass
    import concourse.tile as tile
    from concourse import mybir

    ts, ds = bass.ts, bass.ds
    bf16 = mybir.dt.bfloat16
    f8 = mybir.dt.float8e4
    f16 = mybir.dt.float16
    f32 = mybir.dt.float32
    DR = mybir.MatmulPerfMode.DoubleRow

    nc = bacc.Bacc("TRN2", target_bir_lowering=False, debug=False)
    ansT = nc.dram_tensor("ansT", [EMB, B], bf16, kind="ExternalInput")
    # ans8[p, i, b] = (ans * ANS8_SCALE).T[i*128+p, b]; slicing [:, 2g:2g+2, m]
    # gives the [128, 2, 128] DoubleRow stationary tile for K-pair g
    ans8 = nc.dram_tensor("ans8", [128, KCH, B], f8, kind="ExternalInput")
    # bf16 columns: group 0's trailing 4 tiles, then groups 1..5
    entT = nc.dram_tensor("entT", [EMB, BF_COLS], bf16, kind="ExternalInput")
    # ent8_*[g, p, j, n] = (ent_cols * ENT8_SCALE)[g*256 + j*128 + p, n]
    ent8a = nc.dram_tensor("ent8a", [KPAIRS, 128, KPAIRS, A8_COLS], f8,
                           kind="ExternalInput")
    ent8b = nc.dram_tensor("ent8b", [KPAIRS, 128, KPAIRS, GN], f8,
                           kind="ExternalInput")
    score = nc.dram_tensor("score", [B, SHARD_PAD], f16, kind="ExternalOutput")

    with tile.TileContext(nc) as tc:
        with tc.tile_pool(name="const", bufs=1) as const_pool, \
             tc.tile_pool(name="entp", bufs=10) as ent_pool, \
             tc.tile_pool(name="ent8ap", bufs=KPAIRS) as ent8a_pool, \
             tc.tile_pool(name="ent8bp", bufs=KPAIRS) as ent8b_pool, \
             tc.tile_pool(name="outp", bufs=3) as out_pool, \
             tc.tile_pool(name="outa", bufs=8) as outa_pool, \
             tc.tile_pool(name="ps", bufs=8, space="PSUM") as psum_pool:

            # gpsimd (Pool) cannot read PSUM on TRN2 — copyback on DVE + Act
            ci = 0

            def copyback(dst, src):
                nonlocal ci
                ci += 1
                if ci % 2:
                    nc.vector.tensor_copy(out=dst, in_=src)
                else:
                    nc.scalar.copy(dst, src)

            # ---- startup: issue DMAs in the (g0, m0) block's t-outer
            # consume order — fp8 tiles t0-t2 first (small), then ansT and
            # the bf16 tiles — alternating the two hardware queues.
            ans8_sb = const_pool.tile([128, KCH, B], f8, name="ans8_sb")
            ansT_sb = const_pool.tile([128, KCH, B], bf16, name="ansT_sb")
            e8a = [ent8a_pool.tile([128, KPAIRS, A8_COLS], f8, name="e8a_sb")
                   for _ in range(KPAIRS)]
            ent_sb0 = [ent_pool.tile([128, NBF0 * NTILE], bf16, name="ent_sb")
                       for _ in range(KCH)]
            for gp in range(KPAIRS):
                nc.sync.dma_start(ans8_sb[:, 2 * gp:2 * gp + 2],
                                  ans8[:, 2 * gp:2 * gp + 2])
                nc.scalar.dma_start(e8a[gp][:, :, :NTILE],
                                    ent8a[gp, :, :, :NTILE])
            qi = 0
            for tt in range(1, T8A):
                for gp in range(KPAIRS):
                    eng = nc.sync if qi % 2 else nc.scalar
                    qi += 1
                    eng.dma_start(e8a[gp][:, :, ts(tt, NTILE)],
                                  ent8a[gp, :, :, ds(tt * NTILE, NTILE)])
            for k in range(KCH):
                eng = nc.sync if k % 2 else nc.scalar
                eng.dma_start(ansT_sb[:, k], ansT[ts(k, 128), :])
            for tt in range(NBF0):
                for k in range(KCH):
                    eng = nc.sync if qi % 2 else nc.scalar
                    qi += 1
                    eng.dma_start(ent_sb0[k][:, ts(tt, NTILE)],
                                  entT[ts(k, 128), ds(tt * NTILE, NTILE)])

            def load_group(cols, split=False):
                # one tile per k-chunk so a matmul only waits for its own DMA;
                # split=True rides half the chunks on the Act hw queue (only
                # safe before any output DMA has been queued there)
                tiles = []
                for k in range(KCH):
                    t = ent_pool.tile([128, GN], bf16, name="ent_sb")
                    eng = nc.scalar if (split and k >= 2) else nc.sync
                    eng.dma_start(t[:], entT[ts(k, 128), ds(cols, GN)])
                    tiles.append(t)
                return tiles

            ent_tiles = {1: load_group(NBF0 * NTILE, split=True)}

            def dr_mm(pss_t, m, e8, toff, t, gp):
                nc.tensor.matmul(
                    pss_t[:],
                    ans8_sb[:, 2 * gp:2 * gp + 2, ts(m, 128)],
                    e8[gp][:, :, ds((toff + t) * NTILE, NTILE)],
                    start=(gp == 0),
                    stop=(gp == KPAIRS - 1),
                    perf_mode=DR,
                )

            # ---- group 0: 3 fp8 tiles + 4 bf16 tiles per block
            for m in range(MCH):
                pss = [psum_pool.tile([128, NTILE], f32, name="pst")
                       for _ in range(TPG)]
                if m == 0:
                    # warmup: t-outer streaming in DMA arrival order
                    for t in range(T8A):
                        for gp in range(KPAIRS):
                            dr_mm(pss[t], m, e8a, 0, t, gp)
                        o = outa_pool.tile([128, NTILE], f16, name="outa_sb")
                        copyback(o[:], pss[t][:])
                        eng = nc.sync if t % 2 else nc.scalar
                        eng.dma_start(score[ts(m, 128), ds(t * NTILE, NTILE)],
                                      o[:])
                    for t in range(NBF0):
                        for k in range(KCH):
                            nc.tensor.matmul(
                                pss[T8A + t][:],
                                ansT_sb[:, k, ts(m, 128)],
                                ent_sb0[k][:, ts(t, NTILE)],
                                start=(k == 0),
                                stop=(k == KCH - 1),
                            )
                        o = outa_pool.tile([128, NTILE], f16, name="outa_sb")
                        copyback(o[:], pss[T8A + t][:])
                        eng = nc.sync if t % 2 else nc.scalar
                        eng.dma_start(
                            score[ts(m, 128), ds(A8_COLS + t * NTILE, NTILE)],
                            o[:])
                else:
                    for gp in range(KPAIRS):
                        for t in range(T8A):
                            dr_mm(pss[t], m, e8a, 0, t, gp)
                    for k in range(KCH):
                        lhsT = ansT_sb[:, k, ts(m, 128)]
                        for t in range(NBF0):
                            nc.tensor.matmul(
                                pss[T8A + t][:],
                                lhsT,
                                ent_sb0[k][:, ts(t, NTILE)],
                                start=(k == 0),
                                stop=(k == KCH - 1),
                            )
                    out_sb = out_pool.tile([128, GN], f16, name="out_sb")
                    for t in range(TPG):
                        copyback(out_sb[:, ts(t, NTILE)], pss[t][:])
                    h0 = 4 * NTILE
                    nc.scalar.dma_start(score[ts(m, 128), ds(0, h0)],
                                        out_sb[:, :h0])
                    nc.sync.dma_start(score[ts(m, 128), ds(h0, GN - h0)],
                                      out_sb[:, h0:])

            # ---- groups 1..5: bf16, k-outer / t-inner per m-block
            for g in range(1, 6):
                if g < 5:
                    ent_tiles[g + 1] = load_group(NBF0 * NTILE + g * GN)
                else:
                    # group 6's fp8 tiles (prefetched during group 5)
                    e8b = []
                    for gp in range(KPAIRS):
                        t8v = ent8b_pool.tile([128, KPAIRS, GN], f8,
                                              name="e8b_sb")
                        nc.sync.dma_start(t8v[:], ent8b[gp])
                        e8b.append(t8v)
                ent_sb = ent_tiles.pop(g)
                col0 = g * GN
                for m in range(MCH):
                    pss = [psum_pool.tile([128, NTILE], f32, name="pst")
                           for _ in range(TPG)]
                    out_sb = out_pool.tile([128, GN], f16, name="out_sb")
                    for k in range(KCH):
                        lhsT = ansT_sb[:, k, ts(m, 128)]
                        for t in range(TPG):
                            nc.tensor.matmul(
                                pss[t][:],
                                lhsT,
                                ent_sb[k][:, ts(t, NTILE)],
                                start=(k == 0),
                                stop=(k == KCH - 1),
                            )
                    for t in range(TPG):
                        copyback(out_sb[:, ts(t, NTILE)], pss[t][:])
                    h0 = 4 * NTILE
                    nc.scalar.dma_start(score[ts(m, 128), ds(col0, h0)],
                                        out_sb[:, :h0])
                    nc.sync.dma_start(
                        score[ts(m, 128), ds(col0 + h0, GN - h0)],
                        out_sb[:, h0:])

            # ---- group 6: all fp8 DoubleRow; last block streams per-tile
            col0 = 6 * GN
            for m in range(MCH):
                pss = [psum_pool.tile([128, NTILE], f32, name="pst")
                       for _ in range(TPG)]
                if m < MCH - 1:
                    out_sb = out_pool.tile([128, GN], f16, name="out_sb")
                    for gp in range(KPAIRS):
                        for t in range(TPG):
                            dr_mm(pss[t], m, e8b, 0, t, gp)
                    for t in range(TPG):
                        copyback(out_sb[:, ts(t, NTILE)], pss[t][:])
                    h0 = 4 * NTILE
                    nc.scalar.dma_start(score[ts(m, 128), ds(col0, h0)],
                                        out_sb[:, :h0])
                    nc.sync.dma_start(
                        score[ts(m, 128), ds(col0 + h0, GN - h0)],
                        out_sb[:, h0:])
                else:
                    # t-outer + per-tile DMAs alternating across both hw
                    # queues so the post-matmul tail is tiny
                    for t in range(TPG):
                        for gp in range(KPAIRS):
                            dr_mm(pss[t], m, e8b, 0, t, gp)
                        o = outa_pool.tile([128, NTILE], f16, name="outa_sb")
                        copyback(o[:], pss[t][:])
                        eng = nc.sync if t % 2 else nc.scalar
                        eng.dma_start(
                            score[ts(m, 128), ds(col0 + t * NTILE, NTILE)],
                            o[:])
    nc.compile()
    return nc


def _get_nc():
    global _NC
    if _NC is None:
        _NC = _build_nc()
    return _NC


def _pmap(fn, n):
    from concurrent.futures import ThreadPoolExecutor
    with ThreadPoolExecutor(max_workers=n) as ex:
        list(ex.map(fn, range(n)))


def _fp8_pairs(cols_f32):
    """[512, N] f32 -> [2, 128, 2, N] e4m3 DoubleRow pair layout."""
    n = cols_f32.shape[1]
    q = (cols_f32 * np.float32(ENT8_SCALE)).astype(ml_dtypes.float8_e4m3)
    return np.ascontiguousarray(
        q.reshape(KPAIRS, KPAIRS, 128, n).transpose(0, 2, 1, 3))


def prepare_in_maps(triples, ent_emb, rel_emb):
    triples = np.asarray(triples)
    ent_emb = np.asarray(ent_emb, dtype=np.float32)
    rel_emb = np.asarray(rel_emb, dtype=np.float32)

    d = EMB // 2
    h = ent_emb[triples[:, 0].astype(np.int64)]
    r = rel_emb[triples[:, 1].astype(np.int64)]
    re_h, im_h = h[:, :d], h[:, d:]
    re_r, im_r = r[:, :d], r[:, d:]
    ans = np.empty((B, EMB), np.float32)
    ans[:, :d] = re_h * re_r - im_h * im_r
    ans[:, d:] = re_h * im_r + im_h * re_r
    ansT = np.ascontiguousarray(ans.T)          # [EMB, B] f32, unscaled
    ansT_bf = (ansT * np.float32(OUT_SCALE)).astype(ml_dtypes.bfloat16)
    # DoubleRow stationary layout: [p, i, b] = ansT[i*128 + p, b]
    ans8_q = (ansT * np.float32(ANS8_SCALE)).astype(ml_dtypes.float8_e4m3)
    ans8 = np.ascontiguousarray(
        ans8_q.reshape(KCH, 128, B).transpose(1, 0, 2))

    bf_shards = np.empty((NCORES, EMB, BF_COLS), ml_dtypes.bfloat16)
    f8a_shards = np.empty((NCORES, KPAIRS, 128, KPAIRS, A8_COLS),
                          ml_dtypes.float8_e4m3)
    f8b_shards = np.empty((NCORES, KPAIRS, 128, KPAIRS, GN),
                          ml_dtypes.float8_e4m3)

    def _shard(c):
        sh = np.zeros((EMB, SHARD_PAD), np.float32)
        sh[:, :SHARD] = ent_emb[c * SHARD:(c + 1) * SHARD].T
        f8a_shards[c] = _fp8_pairs(sh[:, :A8_COLS])
        bf_shards[c] = sh[:, A8_COLS:A8_COLS + BF_COLS].astype(
            ml_dtypes.bfloat16)
        f8b_shards[c] = _fp8_pairs(sh[:, 6 * GN:])

    _pmap(_shard, NCORES)
    return [{"ansT": ansT_bf, "ans8": ans8, "entT": bf_shards[c],
             "ent8a": f8a_shards[c], "ent8b": f8b_shards[c]}
            for c in range(NCORES)]


def run_raw(in_maps, trace=False):
    from concourse import bass_utils
    return bass_utils.run_bass_kernel_spmd(
        _get_nc(), in_maps, core_ids=list(range(NCORES)), trace=trace
    )


def assemble(results):
    out = np.empty((B, NUM_ENT), np.float32)
    inv = np.float32(1.0 / OUT_SCALE)
    inv8 = np.float32(1.0 / (OUT_SCALE * FP8_EXTRA))

    def _one(c):
        sh = results[c]["score"][:, :SHARD].astype(np.float32)
        sh[:, :A8_COLS] *= inv8                 # group 0 head: fp8
        sh[:, A8_COLS:A8_COLS + BF_COLS] *= inv   # bf16 columns
        sh[:, A8_COLS + BF_COLS:] *= inv8       # group 6: fp8
        out[:, c * SHARD:(c + 1) * SHARD] = sh

    _pmap(_one, NCORES)
    return out


def kernel(triples, ent_emb, rel_emb):
    in_maps = prepare_in_maps(triples, ent_emb, rel_emb)
    res = run_raw(in_maps)
    return assemble(res.results)


# revision 9
# speedup vs baseline: 1.0181x; 1.0181x over previous
"""ComplEx scoring kernel for 8 Trainium2 NeuronCores.

Math: score[b, e] = Re(<h_b * r_b, conj(ent_e)>) with h = ent_emb[triples[:,0]],
r = rel_emb[triples[:,1]].  Writing ans_b = concat(re_h*re_r - im_h*im_r,
re_h*im_r + im_h*re_r) (shape [B, 512]), the score is exactly
score = ans @ ent_emb.T  — one [1024, 512] x [512, 200000] GEMM.

Strategy (vocab/tensor parallel along the entity axis):
  - host: tiny gather + complex multiply -> ans  (microseconds)
  - shard ent_emb rows 8 ways (25000/core, zero-padded to 25088 = 49*512),
    pre-transposed + cast on host so the device streams contiguous
    [K=512, E] tiles
  - each core: score_shard[1024, 25088] = ansT.T @ entT on the PE array.
    The kernel is PE-bound, so 10 of the 49 column tiles run in fp8e4
    DoubleRow perf mode (2 fp8 K-rows per cycle — halves those columns'
    matmul time; ~3.8% quantization noise on 20% of columns puts the
    global rel err at ~1.7e-2, inside the 2e-2 budget).  The fp8 tiles
    sit at the two ends: 3 tiles open group 0 (lighter warmup stream)
    and group 6 is all fp8 (its blocks run after the input queues have
    drained).  bf16 elsewhere, fp32 PSUM accumulate everywhere.
  - DMA plumbing: a handful of bulk input DMAs (instruction issue on the
    engines costs ~0.6 us apiece, so fewer, bigger transfers win) split
    across the SP and Act hardware queues during warmup; steady-state
    inputs on SP, outputs split across both; the last block streams
    per-512-column outputs so the post-matmul tail is ~1 us
  - host: concatenate the 8 column slabs, unscale, drop padding
"""

import numpy as np
import ml_dtypes

NCORES = 8
NUM_ENT = 200000
EMB = 512
B = 1024
SHARD = NUM_ENT // NCORES      # 25000 entities per core
NTILE = 512                    # matmul moving free dim == one PSUM bank
TPG = 7                        # 512-tiles per DMA group
GN = NTILE * TPG               # 3584 entities per group
NGROUPS = 7
SHARD_PAD = GN * NGROUPS       # 25088
KCH = EMB // 128               # 4 contraction chunks
KPAIRS = 2                     # 2 x (K=256) DoubleRow steps cover K=512
MCH = B // 128                 # 8 batch chunks
T8A = 3                        # leading tiles of group 0 in fp8
NBF0 = TPG - T8A               # group 0's bf16 tiles
A8_COLS = T8A * NTILE          # 1536 fp8 columns at the front
BF_COLS = NBF0 * NTILE + 5 * GN   # bf16 columns: g0 tail + groups 1-5

_NC = None

# score values are ~1e-5 — subnormal in fp16.  Pre-scaling ans by 2**16 on
# the host puts the device-side scores in fp16's normal range, so the output
# can be stored/DMA'd as fp16 (half the write traffic); the host unscales.
OUT_SCALE = 2.0 ** 16
# fp8 operands get extra power-of-2 gain to sit comfortably inside e4m3's
# +-240 range: ans * 2**17 (abs max ~190), ent * 2**11 (abs max ~36).  The
# fp8 columns' scores come out 2**12 hotter than the bf16 ones (max ~2e4,
# still inside f16); assemble() divides that back out.
ANS8_SCALE = 2.0 ** 17
ENT8_SCALE = 2.0 ** 11
FP8_EXTRA = ANS8_SCALE * ENT8_SCALE / OUT_SCALE


def _build_nc():
    import concourse.bacc as bacc
    import concourse.bass as bass
    import concourse.tile as tile
    from concourse import mybir

    ts, ds = bass.ts, bass.ds
    bf16 = mybir.dt.bfloat16
    f8 = mybir.dt.float8e4
    f16 = mybir.dt.float16
    f32 = mybir.dt.float32
    DR = mybir.MatmulPerfMode.DoubleRow

    nc = bacc.Bacc("TRN2", target_bir_lowering=False, debug=False)
    # ansT/ans8[p, i, b] = (ans * scale).T[i*128+p, b]: chunk-pair layout so
    # one DMA loads everything and [:, 2g:2g+2, m] is a DoubleRow tile
    ansT = nc.dram_tensor("ansT", [128, KCH, B], bf16, kind="ExternalInput")
    ans8 = nc.dram_tensor("ans8", [128, KCH, B], f8, kind="ExternalInput")
    # bf16 columns: group 0's trailing 4 tiles, then groups 1..5
    entT = nc.dram_tensor("entT", [EMB, BF_COLS], bf16, kind="ExternalInput")
    # ent8_*[g, p, j, n] = (ent_cols * ENT8_SCALE)[g*256 + j*128 + p, n]
    ent8a = nc.dram_tensor("ent8a", [KPAIRS, 128, KPAIRS, A8_COLS], f8,
                           kind="ExternalInput")
    ent8b = nc.dram_tensor("ent8b", [KPAIRS, 128, KPAIRS, GN], f8,
                           kind="ExternalInput")
    score = nc.dram_tensor("score", [B, SHARD_PAD], f16, kind="ExternalOutput")

    with tile.TileContext(nc) as tc:
        with tc.tile_pool(name="const", bufs=1) as const_pool, \
             tc.tile_pool(name="ent0p", bufs=KCH) as ent0_pool, \
             tc.tile_pool(name="entp", bufs=2 * KCH) as ent_pool, \
             tc.tile_pool(name="ent8ap", bufs=KPAIRS) as ent8a_pool, \
             tc.tile_pool(name="ent8bp", bufs=KPAIRS) as ent8b_pool, \
             tc.tile_pool(name="outp", bufs=4) as out_pool, \
             tc.tile_pool(name="outa", bufs=7) as outa_pool, \
             tc.tile_pool(name="ps", bufs=8, space="PSUM") as psum_pool:

            # gpsimd (Pool) cannot read PSUM on TRN2 — copyback on DVE + Act
            ci = 0

            def copyback(dst, src):
                nonlocal ci
                ci += 1
                if ci % 2:
                    nc.vector.tensor_copy(out=dst, in_=src)
                else:
                    nc.scalar.copy(dst, src)

            # ---- startup: few bulk DMAs, split across both hardware queues
            # in the (g0, m0) block's consume order (fp8 tiles first).
            ans8_sb = const_pool.tile([128, KCH, B], f8, name="ans8_sb")
            ansT_sb = const_pool.tile([128, KCH, B], bf16, name="ansT_sb")
            e8a = [ent8a_pool.tile([128, KPAIRS, A8_COLS], f8, name="e8a_sb")
                   for _ in range(KPAIRS)]
            ent_sb0 = [ent0_pool.tile([128, NBF0 * NTILE], bf16,
                                      name="ent0_sb") for _ in range(KCH)]
            nc.sync.dma_start(ans8_sb[:], ans8[:])
            nc.scalar.dma_start(e8a[1][:], ent8a[1])
            nc.sync.dma_start(e8a[0][:], ent8a[0])
            nc.scalar.dma_start(ansT_sb[:], ansT[:])
            for k in range(KCH):
                eng = nc.sync if k % 2 == 0 else nc.scalar
                eng.dma_start(ent_sb0[k][:],
                              entT[ts(k, 128), :NBF0 * NTILE])

            def load_group(cols, split=False):
                # one tile per k-chunk so a matmul only waits for its own DMA;
                # split=True rides half the chunks on the Act hw queue (only
                # safe before any output DMA has been queued there)
                tiles = []
                for k in range(KCH):
                    t = ent_pool.tile([128, GN], bf16, name="ent_sb")
                    eng = nc.scalar if (split and k % 2) else nc.sync
                    eng.dma_start(t[:], entT[ts(k, 128), ds(cols, GN)])
                    tiles.append(t)
                return tiles

            ent_tiles = {1: load_group(NBF0 * NTILE, split=True)}

            def dr_mm(pss_t, m, e8, t, gp):
                nc.tensor.matmul(
                    pss_t[:],
                    ans8_sb[:, 2 * gp:2 * gp + 2, ts(m, 128)],
                    e8[gp][:, :, ts(t, NTILE)],
                    start=(gp == 0),
                    stop=(gp == KPAIRS - 1),
                    perf_mode=DR,
                )

            def out_block(m, col0, pss, n):
                out_sb = out_pool.tile([128, n * NTILE], f16, name="out_sb")
                for t in range(n):
                    copyback(out_sb[:, ts(t, NTILE)], pss[t][:])
                h0 = (n // 2 + 1) * NTILE
                e1, e2 = (nc.scalar, nc.sync) if m % 2 else (nc.sync, nc.scalar)
                e1.dma_start(score[ts(m, 128), ds(col0, h0)], out_sb[:, :h0])
                e2.dma_start(score[ts(m, 128), ds(col0 + h0, n * NTILE - h0)],
                             out_sb[:, h0:])

            # ---- group 0: 3 fp8 tiles + 4 bf16 tiles per block
            for m in range(MCH):
                pss = [psum_pool.tile([128, NTILE], f32, name="pst")
                       for _ in range(TPG)]
                for gp in range(KPAIRS):
                    for t in range(T8A):
                        dr_mm(pss[t], m, e8a, t, gp)
                for k in range(KCH):
                    lhsT = ansT_sb[:, k, ts(m, 128)]
                    for t in range(NBF0):
                        nc.tensor.matmul(
                            pss[T8A + t][:],
                            lhsT,
                            ent_sb0[k][:, ts(t, NTILE)],
                            start=(k == 0),
                            stop=(k == KCH - 1),
                        )
                out_block(m, 0, pss, TPG)

            # ---- groups 1..5: bf16, k-outer / t-inner per m-block
            for g in range(1, 6):
                if g < 5:
                    ent_tiles[g + 1] = load_group(NBF0 * NTILE + g * GN)
                else:
                    # group 6's fp8 tiles (prefetched during group 5)
                    e8b = []
                    for gp in range(KPAIRS):
                        t8v = ent8b_pool.tile([128, KPAIRS, GN], f8,
                                              name="e8b_sb")
                        nc.sync.dma_start(t8v[:], ent8b[gp])
                        e8b.append(t8v)
                ent_sb = ent_tiles.pop(g)
                col0 = g * GN
                for m in range(MCH):
                    pss = [psum_pool.tile([128, NTILE], f32, name="pst")
                           for _ in range(TPG)]
                    for k in range(KCH):
                        lhsT = ansT_sb[:, k, ts(m, 128)]
                        for t in range(TPG):
                            nc.tensor.matmul(
                                pss[t][:],
                                lhsT,
                                ent_sb[k][:, ts(t, NTILE)],
                                start=(k == 0),
                                stop=(k == KCH - 1),
                            )
                    out_block(m, col0, pss, TPG)

            # ---- group 6: all fp8 DoubleRow; last block streams per-tile
            col0 = 6 * GN
            for m in range(MCH):
                pss = [psum_pool.tile([128, NTILE], f32, name="pst")
                       for _ in range(TPG)]
                if m < MCH - 1:
                    for gp in range(KPAIRS):
                        for t in range(TPG):
                            dr_mm(pss[t], m, e8b, t, gp)
                    out_block(m, col0, pss, TPG)
                else:
                    # t-outer + per-tile DMAs alternating across both hw
                    # queues so the post-matmul tail is tiny
                    for t in range(TPG):
                        for gp in range(KPAIRS):
                            dr_mm(pss[t], m, e8b, t, gp)
                        o = outa_pool.tile([128, NTILE], f16, name="outa_sb")
                        copyback(o[:], pss[t][:])
                        eng = nc.sync if t % 2 else nc.scalar
                        eng.dma_start(
                            score[ts(m, 128), ds(col0 + t * NTILE, NTILE)],
                            o[:])
    nc.compile()
    return nc


def _get_nc():
    global _NC
    if _NC is None:
        _NC = _build_nc()
    return _NC


def _pmap(fn, n):
    from concurrent.futures import ThreadPoolExecutor
    with ThreadPoolExecutor(max_workers=n) as ex:
        list(ex.map(fn, range(n)))


def _fp8_pairs(cols_f32):
    """[512, N] f32 -> [2, 128, 2, N] e4m3 DoubleRow pair layout."""
    n = cols_f32.shape[1]
    q = (cols_f32 * np.float32(ENT8_SCALE)).astype(ml_dtypes.float8_e4m3)
    return np.ascontiguousarray(
        q.reshape(KPAIRS, KPAIRS, 128, n).transpose(0, 2, 1, 3))


def prepare_in_maps(triples, ent_emb, rel_emb):
    triples = np.asarray(triples)
    ent_emb = np.asarray(ent_emb, dtype=np.float32)
    rel_emb = np.asarray(rel_emb, dtype=np.float32)

    d = EMB // 2
    h = ent_emb[triples[:, 0].astype(np.int64)]
    r = rel_emb[triples[:, 1].astype(np.int64)]
    re_h, im_h = h[:, :d], h[:, d:]
    re_r, im_r = r[:, :d], r[:, d:]
    ans = np.empty((B, EMB), np.float32)
    ans[:, :d] = re_h * re_r - im_h * im_r
    ans[:, d:] = re_h * im_r + im_h * re_r
    ansT = np.ascontiguousarray(ans.T)          # [EMB, B] f32, unscaled
    # chunk-pair layout [p, i, b] = ansT[i*128 + p, b]
    ansT_bf = np.ascontiguousarray(
        (ansT * np.float32(OUT_SCALE)).astype(ml_dtypes.bfloat16)
        .reshape(KCH, 128, B).transpose(1, 0, 2))
    ans8 = np.ascontiguousarray(
        (ansT * np.float32(ANS8_SCALE)).astype(ml_dtypes.float8_e4m3)
        .reshape(KCH, 128, B).transpose(1, 0, 2))

    bf_shards = np.empty((NCORES, EMB, BF_COLS), ml_dtypes.bfloat16)
    f8a_shards = np.empty((NCORES, KPAIRS, 128, KPAIRS, A8_COLS),
                          ml_dtypes.float8_e4m3)
    f8b_shards = np.empty((NCORES, KPAIRS, 128, KPAIRS, GN),
                          ml_dtypes.float8_e4m3)

    def _shard(c):
        sh = np.zeros((EMB, SHARD_PAD), np.float32)
        sh[:, :SHARD] = ent_emb[c * SHARD:(c + 1) * SHARD].T
        f8a_shards[c] = _fp8_pairs(sh[:, :A8_COLS])
        bf_shards[c] = sh[:, A8_COLS:A8_COLS + BF_COLS].astype(
            ml_dtypes.bfloat16)
        f8b_shards[c] = _fp8_pairs(sh[:, 6 * GN:])

    _pmap(_shard, NCORES)
    return [{"ansT": ansT_bf, "ans8": ans8, "entT": bf_shards[c],
             "ent8a": f8a_shards[c], "ent8b": f8b_shards[c]}
            for c in range(NCORES)]


def run_raw(in_maps, trace=False):
    from concourse import bass_utils
    return bass_utils.run_bass_kernel_spmd(
        _get_nc(), in_maps, core_ids=list(range(NCORES)), trace=trace
    )


def assemble(results):
    out = np.empty((B, NUM_ENT), np.float32)
    inv = np.float32(1.0 / OUT_SCALE)
    inv8 = np.float32(1.0 / (OUT_SCALE * FP8_EXTRA))

    def _one(c):
        sh = results[c]["score"][:, :SHARD].astype(np.float32)
        sh[:, :A8_COLS] *= inv8                 # group 0 head: fp8
        sh[:, A8_COLS:A8_COLS + BF_COLS] *= inv   # bf16 columns
        sh[:, A8_COLS + BF_COLS:] *= inv8       # group 6: fp8
        out[:, c * SHARD:(c + 1) * SHARD] = sh

    _pmap(_one, NCORES)
    return out


def kernel(triples, ent_emb, rel_emb):
    in_maps = prepare_in_maps(triples, ent_emb, rel_emb)
    res = run_raw(in_maps)
    return assemble(res.results)
